# revision 14
# baseline (speedup 1.0000x reference)
"""AdaptiveGraphAttention Trainium2 kernel (8 NeuronCores, data-parallel).

Math: in the reference, logits[b,h,i,j] = a_q[b,h,i] + a_k[b,h,j] +
e_j[b,h,j]*adj[i,j] + attn_b with adj[:,0]=0, adj[:,1:]=1 — the mask and the
j-dependent terms are identical for every query row i, and the a_q/bias terms
are constant over j.  Softmax is shift-invariant, so the attention
distribution p[b,h,:] = softmax_{j>=1}(a_k + e_j) is the same for all i: the
attention matrix is rank-1 and the output is one row per batch, broadcast
over the 256 query positions.  bq/bk/attn_b cancel exactly; bv survives as
an additive constant (sum_j p_j = 1); bv and bo are folded on the host.

Per-head dots fold into small matrices:
  a_k[b,j,h] = nv[b,j,:] @ Uk[:,h],  Uk[d,h] = sum_m Wk[h*64+m, d] * w_k[m]
  e_j[b,j,h] = desc[b,j-1,:] @ Ue[:,h], Ue[h*64+m, h] = w_e(m) (else 0)

Device work per core (4 batches); fp8e4m3 logits path (inputs scaled by
USCALE to clear the fp8 subnormal floor, undone by the exp's scale), bf16
everywhere else, f32 PSUM accumulation:
  c[h,j]    = Uk.T @ nvT[:, j] + Ue.T @ descT[:, j-1]  (PE DoubleRow fp8)
  p[h,:]    = softmax_j(c)   (no max-subtraction: logits are O(1));
              the 1/sum normalization is folded into the p-transpose by
              multiplying with diag(recip) on the PE.
  nvbarT    = nv.T @ pT           [1024, 16] per batch      (PE, one PSUM
              tile for all 8 d-chunks -> single copy to SBUF)
  VbarT     = WvT.T @ nvbarT_all  [1024, 4*16] per d'-chunk (PE)
  ctxT      = blockdiag-select(VbarT)        [1024, 4]      (DVE)
  out       = ctxT.T @ WoT        [4, 1024]                 (PE) -> DMA
bv is folded into the host-side output bias (out += Wo @ bv + bo, exact
since sum_j p_j = 1).

Schedule: the kernel is HBM-DMA-bound (~8.4MB/core at ~358GB/s).  All DMAs
issue up-front on the sync queue in consumption order: per-batch activations
first (batch 0's xT split so the first DR matmul starts ~1us after the first
bytes land), then WvT in cm-halves, then WoT in output-column halves — the
Vbar and out stages chase the incoming weight stream instead of running as a
serial tail after it.

All DRAM inputs are host-prepermuted to [128, chunk, inner] so each DMA
partition row is one contiguous run (descriptor-count relief).
"""

import numpy as np
import ml_dtypes
from contextlib import ExitStack

import concourse.bass as bass
import concourse.mybir as mybir
import concourse.tile as tile
from concourse import bacc
from concourse.bass_utils import run_bass_kernel_spmd
from concourse.masks import make_identity

B, S, D, H, HD = 32, 256, 1024, 16, 64
NCORES = 8
BPC = B // NCORES  # 4 batches per core
F32 = mybir.dt.float32
BF16 = mybir.dt.bfloat16
NPBF = ml_dtypes.bfloat16
F8 = mybir.dt.float8e4
NPF8 = ml_dtypes.float8_e4m3
USCALE = 512.0  # fp8 range lift for the tiny folded U entries
DC = D // 128  # 8 chunks of the model dim
JC = S // 128  # 2 chunks of the sequence dim

_cache = {}


def _build():
    nc = bacc.Bacc("TRN2", target_bir_lowering=False, debug=False,
                   num_devices=NCORES)

    nv_ext = nc.declare_dram_parameter("nv", [BPC, 128, JC, D], BF16,
                                       isOutput=False)
    xt_ext = nc.declare_dram_parameter("xT", [BPC, 128, DC, 2 * S], F8,
                                       isOutput=False)
    u_ext = nc.declare_dram_parameter("U", [128, DC, 2 * H], F8,
                                      isOutput=False)
    wvt_ext = nc.declare_dram_parameter("WvT", [128, DC, DC, 128], BF16,
                                        isOutput=False)
    wot_ext = nc.declare_dram_parameter("WoT", [128, DC, DC, 128], BF16,
                                        isOutput=False)
    out_ext = nc.declare_dram_parameter("out", [128, DC, BPC], F32,
                                        isOutput=True)

    with tile.TileContext(nc) as tc, ExitStack() as ctx:
        wpool = ctx.enter_context(tc.tile_pool(name="w", bufs=1))
        xpool = ctx.enter_context(tc.tile_pool(name="x", bufs=4))
        smpool = ctx.enter_context(tc.tile_pool(name="sm", bufs=2))
        pspool = ctx.enter_context(tc.tile_pool(name="ps", bufs=2,
                                                space="PSUM"))

        # --- resident constants -------------------------------------------
        ident = wpool.tile([128, 128], F32)
        make_identity(nc, ident[:])
        u_sb = wpool.tile([128, DC, 2 * H], F8)
        # U goes on the gpsimd queue so it doesn't take a slot ahead of the
        # batch-0 activations on the sync queue.
        nc.gpsimd.dma_start(out=u_sb[:], in_=u_ext.ap())

        nvall = wpool.tile([128, DC, BPC * H], BF16)  # nvbarT, all batches
        ctx_sb = wpool.tile([128, DC, BPC], BF16)
        wvt_sb = wpool.tile([128, DC, DC, 128], BF16)
        wot_sb = wpool.tile([128, DC, DC, 128], BF16)

        # --- all input DMAs up-front, in consumption order ----------------
        xt_tiles, nv_tiles = [], []
        for b in range(BPC):
            xt_sb = xpool.tile([128, DC, 2 * S], F8, tag="xt")
            if b == 0:
                nc.sync.dma_start(out=xt_sb[:, 0:2], in_=xt_ext[b, :, 0:2])
                nc.sync.dma_start(out=xt_sb[:, 2:DC], in_=xt_ext[b, :, 2:DC])
            else:
                nc.sync.dma_start(out=xt_sb[:], in_=xt_ext[b])
            nv_sb = xpool.tile([128, JC, D], BF16, tag="nv")
            nc.sync.dma_start(out=nv_sb[:], in_=nv_ext[b])
            xt_tiles.append(xt_sb)
            nv_tiles.append(nv_sb)
        nc.sync.dma_start(out=wvt_sb[:], in_=wvt_ext.ap())
        for sl in (slice(0, 6), slice(6, 8)):
            nc.sync.dma_start(out=wot_sb[:, sl], in_=wot_ext[:, sl])

        # --- batch loop, software-pipelined: the PE stream is
        # L0, L1, tail0, L2, tail1, L3, tail2, tail3 so each batch's softmax
        # chain (scalar/vector) hides behind the next batch's logits on PE --
        def logits(b):
            psc = pspool.tile([H, S - 1], F32, tag="s", name=f"psc{b}")
            xt_sb = xt_tiles[b]
            DR = mybir.MatmulPerfMode.DoubleRow
            for c2 in range(DC // 2):
                pair = slice(2 * c2, 2 * c2 + 2)
                nc.tensor.matmul(psc[:], u_sb[:, pair, 0:H],
                                 xt_sb[:, pair, 1:S],
                                 start=(c2 == 0), stop=False, perf_mode=DR)
            for c2 in range(DC // 2):
                pair = slice(2 * c2, 2 * c2 + 2)
                nc.tensor.matmul(psc[:], u_sb[:, pair, H:2 * H],
                                 xt_sb[:, pair, S:2 * S - 1],
                                 start=False, stop=(c2 == DC // 2 - 1),
                                 perf_mode=DR)
            return psc

        def tail(b, psc):
            # softmax over j (free dim); logits are O(1), no max-subtraction
            nv_sb = nv_tiles[b]
            p_sb = smpool.tile([H, S], BF16, tag="p", name=f"p{b}")
            nc.gpsimd.memset(p_sb[:, 0:1], 0.0)
            sumx = smpool.tile([H, 1], F32, tag="sum", name=f"sumx{b}")
            nc.scalar.activation(p_sb[:, 1:S], psc[:],
                                 mybir.ActivationFunctionType.Exp,
                                 scale=1.0 / USCALE,
                                 accum_out=sumx[:])
            recip = smpool.tile([H, 1], F32, tag="recip", name=f"recip{b}")
            nc.vector.reciprocal(recip[:], sumx[:])
            # diag(recip): normalization rides the transpose matmul for free
            diag = smpool.tile([H, H], BF16, tag="diag", name=f"diag{b}")
            nc.vector.tensor_scalar_mul(diag[:], ident[0:H, 0:H], recip[:])

            # pT[j, h] = p[:, j].T @ diag  (PE), then cast to bf16
            pt_sb = smpool.tile([128, JC, H], BF16, tag="pt", name=f"pt{b}")
            for jc in range(JC):
                pt_ps = pspool.tile([128, H], F32, tag="pt", name=f"ptp{b}",
                                    bufs=1)
                nc.tensor.matmul(pt_ps[:], p_sb[:, jc * 128:(jc + 1) * 128],
                                 diag[:], start=True, stop=True)
                nc.vector.tensor_copy(pt_sb[:, jc, :], pt_ps[:])

            # nvbarT[d, h] for all 8 d-chunks into one PSUM tile, one copy
            nb_ps = pspool.tile([128, DC * H], F32, tag="nb", name=f"nb{b}")
            for cm in range(DC):
                for jc in range(JC):
                    nc.tensor.matmul(nb_ps[:, cm * H:(cm + 1) * H],
                                     nv_sb[:, jc, cm * 128:(cm + 1) * 128],
                                     pt_sb[:, jc, :],
                                     start=(jc == 0), stop=(jc == JC - 1))
            nc.vector.tensor_copy(
                nvall[:, :, b * H:(b + 1) * H],
                nb_ps[:].rearrange("p (c h) -> p c h", h=H))

        psc_live = logits(0)
        for b in range(BPC):
            psc_next = logits(b + 1) if b + 1 < BPC else None
            tail(b, psc_live)
            psc_live = psc_next

        # --- VbarT then blockdiag select, per d'-chunk (chases WvT DMA) ---
        for cm in range(DC):
            vb_ps = pspool.tile([128, BPC * H], F32, tag="vb", bufs=3)
            for ck in range(DC):
                nc.tensor.matmul(vb_ps[:],
                                 wvt_sb[:, cm, ck, :],
                                 nvall[:, ck, :],
                                 start=(ck == 0), stop=(ck == DC - 1))
            for half in range(2):
                h = 2 * cm + half
                rows = slice(64 * half, 64 * half + 64)
                s_ap = vb_ps[rows, :].rearrange("p (b h) -> p b h", h=H)[:, :, h]
                nc.vector.tensor_copy(ctx_sb[rows, cm, :], s_ap)

        # --- outT[e, b] = sum_d' WoT[d', e] ctxT[d', b], per 128-wide
        # e-chunk (chases the WoT DMA; 128-partition output so the final
        # copy+DMA stay on all DVE lanes) ----------------------------------
        o_ps = pspool.tile([128, DC * BPC], F32, tag="s")
        o_sb = smpool.tile([128, DC, BPC], F32, tag="osb")
        for half in range(2):
            for ec in range(4 * half, 4 * half + 4):
                for ck in range(DC):
                    nc.tensor.matmul(o_ps[:, ec * BPC:(ec + 1) * BPC],
                                     wot_sb[:, ec, ck, :],
                                     ctx_sb[:, ck, :],
                                     start=(ck == 0), stop=(ck == DC - 1))
            ecs = slice(4 * half, 4 * half + 4)
            nc.vector.tensor_copy(
                o_sb[:, ecs, :],
                o_ps[:, 16 * half:16 * half + 16].rearrange(
                    "p (e b) -> p e b", b=BPC))
            nc.sync.dma_start(out=out_ext[:, ecs], in_=o_sb[:, ecs, :])

    nc.compile()
    return nc


def _prep(desc, nv, Wk, Wv, Wo, attn_w):
    w_k = attn_w[HD:2 * HD]
    w_e = attn_w[2 * HD:]
    Uk = np.einsum('hmd,m->dh', Wk.reshape(H, HD, D), w_k)
    Ue = np.zeros((D, H), np.float32)
    for h in range(H):
        Ue[h * HD:(h + 1) * HD, h] = w_e
    U = np.concatenate([Uk, Ue], axis=1) * USCALE           # [D, 32]
    Up = np.ascontiguousarray(
        U.reshape(DC, 128, 2 * H).swapaxes(0, 1)).astype(NPF8)
    WvTp = np.ascontiguousarray(
        Wv.T.reshape(DC, 128, DC, 128).transpose(1, 2, 0, 3)).astype(NPBF)
    WoTp = np.ascontiguousarray(
        Wo.T.reshape(DC, 128, DC, 128).transpose(1, 2, 0, 3)).astype(NPBF)
    # nv natural, chunked over j: [B, 128, JC, D]
    nvp = np.ascontiguousarray(
        nv.reshape(B, JC, 128, D).swapaxes(1, 2)).astype(NPBF)
    # nv transposed, chunked over d: [B, 128, DC, S]
    nvTp = nv.transpose(0, 2, 1).reshape(B, DC, 128, S).swapaxes(1, 2)
    descTp = desc.transpose(0, 2, 1).reshape(B, DC, 128, S - 1).swapaxes(1, 2)
    pad = np.zeros((B, 128, DC, 1), np.float32)
    xTp = np.concatenate([nvTp, descTp, pad], axis=3).astype(NPF8)
    return Up, WvTp, WoTp, nvp, xTp


def kernel(desc_embeddings, name_value_embeddings, Wq, bq, Wk, bk, Wv, bv,
           attn_w, attn_b, Wo, bo, _trace=False):
    desc = np.asarray(desc_embeddings, np.float32)
    nv = np.asarray(name_value_embeddings, np.float32)
    Up, WvTp, WoTp, nvp, xTp = _prep(
        desc, nv, np.asarray(Wk, np.float32), np.asarray(Wv, np.float32),
        np.asarray(Wo, np.float32), np.asarray(attn_w, np.float32))

    if "nc" not in _cache:
        _cache["nc"] = _build()
    nc = _cache["nc"]

    in_maps = []
    for c in range(NCORES):
        sl = slice(c * BPC, (c + 1) * BPC)
        in_maps.append({
            "nv": np.ascontiguousarray(nvp[sl]),
            "xT": np.ascontiguousarray(xTp[sl]),
            "U": Up, "WvT": WvTp, "WoT": WoTp,
        })
    res = run_bass_kernel_spmd(nc, in_maps, core_ids=list(range(NCORES)),
                               trace=_trace)
    out_rows = np.empty((B, D), np.float32)
    for c in range(NCORES):
        ot = np.asarray(res.results[c]["out"])  # [128, DC, BPC] = outT
        out_rows[c * BPC:(c + 1) * BPC] = ot.transpose(2, 1, 0).reshape(BPC, D)
    bo_eff = (np.asarray(bo, np.float32)
              + np.asarray(Wo, np.float32) @ np.asarray(bv, np.float32))
    out_rows += bo_eff[None, :]
    full = np.broadcast_to(out_rows[:, None, :], (B, S, D))
    if _trace:
        return np.ascontiguousarray(full), res
    return np.ascontiguousarray(full)
